# revision 1
# baseline (speedup 1.0000x reference)
"""Trainium2 Bass kernel for nn_Channel_Transposed_Attention (B8 C384 H64 W64).

Data-parallel over batch: 8 batch elements -> 8 NeuronCores (SPMD, per-core
x slice). Per core everything lives in (C, N) channel-major layout
(N = H*W), tiled in 96-channel tiles so attention head-pairs, the x1/x2
gate split and all channel tilings align on partitions. q,k are produced in
(N, C) token-major layout; per head the [q_h|k_h] Gram matrix gives both the
attention logits and the l2-norm diagonals in one accumulated matmul chain.
Depthwise convs run as PE tap-accumulation with per-channel diagonal weight
matrices (bf16) over zero-padded row-strided buffers.
"""
import os
import numpy as np
from contextlib import ExitStack

import concourse.bass as bass
import concourse.bacc as bacc
import concourse.tile as tile
from concourse import mybir
from concourse.bass_utils import run_bass_kernel_spmd
from concourse._compat import with_exitstack

import ml_dtypes
BF16 = ml_dtypes.bfloat16

F32 = mybir.dt.float32
F32R = mybir.dt.float32r
BF = mybir.dt.bfloat16
AF = mybir.ActivationFunctionType
OP = mybir.AluOpType
AX = mybir.AxisListType

H = W = 64
N = H * W               # 4096
HP = W + 2              # 66   pad-1 row stride
NP1 = (H + 2) * HP      # 4356
BP = W + 18             # 82   pad-9 row stride (ci2b)
C = 384
C6, C2, C4 = 64, 192, 96
HEADS, HD = 8, 48
NCORES = 8
CH = 512
NCH = N // CH           # 8
QKW = 2 * C             # 768

_last_results = None


def _r(x):
    return x.bitcast(F32R)


def _win(t, off, dims, p=None):
    """Strided free-dim window of a 2D tile AP at free element offset."""
    base = t[:, off:off + 1] if p is None else t[p[0]:p[1], off:off + 1]
    return bass.AP(tensor=base.tensor, offset=base.offset,
                   ap=[list(base.ap[0])] + [list(dd) for dd in dims])


def _pbcast(row_ap, parts):
    """Partition-broadcast a [1, F] AP to [parts, F]."""
    return bass.AP(tensor=row_ap.tensor, offset=row_ap.offset,
                   ap=[[0, parts]] + [list(dd) for dd in row_ap.ap[1:]])


def _diag(wcol, p):
    d = np.zeros((p, p), np.float32)
    d[np.arange(p), np.arange(p)] = wcol
    return d


def build_host_inputs(inputs):
    g = {}
    qkv_w = np.asarray(inputs["qkv_w"], np.float32)
    wtf = qkv_w.T                                    # [384, 1152]
    # v section padded: pair p -> cols [ch 96p..+48 | 16 zero | ch +48..+96 | 16 zero]
    wtv = np.zeros((C, 512), np.float32)
    for p in range(4):
        wtv[:, 128 * p:128 * p + 48] = wtf[:, QKW + 96 * p:QKW + 96 * p + 48]
        wtv[:, 128 * p + 64:128 * p + 112] = wtf[:, QKW + 96 * p + 48:
                                                 QKW + 96 * (p + 1)]
    g["wt"] = np.ascontiguousarray(
        np.concatenate([wtf[:, :QKW], wtv], 1)).astype(BF16)   # [384, 1280]
    dw1T = np.asarray(inputs["dw1_w"], np.float32).reshape(C, C).T
    dw1Tp = np.zeros((4, 128, C), np.float32)
    for k in range(4):
        dw1Tp[k, 0:48] = dw1T[96 * k:96 * k + 48]
        dw1Tp[k, 64:112] = dw1T[96 * k + 48:96 * (k + 1)]
    g["dw1T"] = np.ascontiguousarray(dw1Tp).astype(BF16)
    g["projT"] = np.ascontiguousarray(
        np.asarray(inputs["proj_w"], np.float32).T.reshape(4, 96, C)
    ).astype(BF16)
    g["cpinT"] = np.ascontiguousarray(
        np.asarray(inputs["cp_in_w"], np.float32).reshape(C6, C).T
        .reshape(4, 96, C6)).astype(BF16)
    g["ci1T"] = np.ascontiguousarray(
        np.asarray(inputs["ci1_w"], np.float32).reshape(C6, C6).T)
    ci2cT = np.asarray(inputs["ci2c_w"], np.float32).reshape(C6, C6).T
    z = np.zeros_like(ci2cT)
    g["ci2cT"] = np.ascontiguousarray(
        np.stack([np.vstack([ci2cT, z]), np.vstack([z, ci2cT])])).astype(BF16)
    g["cpoutT"] = np.ascontiguousarray(
        np.asarray(inputs["cp_out_w"], np.float32).reshape(C, C6).T
    ).astype(BF16)
    g["spinT"] = np.ascontiguousarray(
        np.asarray(inputs["sp_in_w"], np.float32).reshape(C2, C).T
        .reshape(4, 96, C2)).astype(BF16)
    g["spoutT"] = np.ascontiguousarray(
        np.asarray(inputs["sp_out_w"], np.float32).reshape(C, C4).T
    ).astype(BF16)

    dw2 = np.asarray(inputs["dw2_w"], np.float32).reshape(C, 9)
    g["dw2_diag"] = np.ascontiguousarray(np.stack([
        np.stack([_diag(dw2[96 * m:96 * m + 96, t], 96) for t in range(9)])
        for m in range(4)])).astype(BF16)
    spdw = np.asarray(inputs["sp_dw_w"], np.float32).reshape(C2, 9)
    g["spdw_diag"] = np.ascontiguousarray(np.stack([
        np.stack([_diag(spdw[96 * m:96 * m + 96, t], 96) for t in range(9)])
        for m in range(2)])).astype(BF16)
    cia = np.asarray(inputs["ci2a_w"], np.float32).reshape(C6, 9)
    g["cia_diag"] = np.ascontiguousarray(np.stack(
        [_diag(np.concatenate([cia[:, t], cia[:, t]]), 128)
         for t in range(9)])).astype(BF16)
    cib = np.asarray(inputs["ci2b_w"], np.float32).reshape(C6, 49)
    g["cib_diag"] = np.ascontiguousarray(np.stack(
        [_diag(np.concatenate([cib[:, t], cib[:, t]]), 128)
         for t in range(49)])).astype(BF16)

    for nm in ["dw1_b", "dw2_b", "cp_in_b", "ci1_b", "ci2c_b", "cp_out_b",
               "sp_in_b", "sp_dw_b", "sp_out_b", "proj_b"]:
        g[nm] = np.asarray(inputs[nm], np.float32)
    g["ci2a_b2"] = np.tile(np.asarray(inputs["ci2a_b"], np.float32), 2)
    g["ci2b_b2"] = np.tile(np.asarray(inputs["ci2b_b"], np.float32), 2)
    g["temp"] = np.ascontiguousarray(
        np.asarray(inputs["temperature"], np.float32).reshape(1, HEADS))
    g["i96"] = np.eye(96, dtype=np.float32)
    g["i48b"] = np.eye(48, dtype=np.float32).astype(BF16)
    return g


@with_exitstack
def emit(ctx: ExitStack, tc, d):
    nc = tc.nc
    sync = nc.sync

    # ---- persistent weights ------------------------------------------------
    wp = ctx.enter_context(tc.tile_pool(name="wp", bufs=1))

    def load2(nm):
        src = d[nm]
        t = wp.tile(list(src.shape), src.dtype, name=f"sb_{nm}")
        sync.dma_start(out=t, in_=src[:])
        return t

    def load3(nm):
        src = d[nm]
        ts = []
        for i in range(src.shape[0]):
            t = wp.tile(list(src.shape[1:]), src.dtype, name=f"sb_{nm}{i}")
            sync.dma_start(out=t, in_=src[i])
            ts.append(t)
        return ts

    def loadb(nm, p0, p):
        t = wp.tile([p, 1], F32, name=f"sb_{nm}_{p0}")
        sync.dma_start(out=t, in_=d[nm][p0:p0 + p].rearrange("(a b) -> a b",
                                                             b=1))
        return t

    dw1T = load3("dw1T")
    projT = load3("projT")
    cpinT = load3("cpinT")
    ci1T = load2("ci1T")
    ci2cT = load3("ci2cT")
    cpoutT = load2("cpoutT")
    spinT = load3("spinT")
    spoutT = load2("spoutT")
    dw2_diag = []
    for m in range(4):
        row = []
        for t_ in range(9):
            tl = wp.tile([96, 96], BF, name=f"dw2d{m}_{t_}")
            sync.dma_start(out=tl, in_=d["dw2_diag"][m, t_])
            row.append(tl)
        dw2_diag.append(row)
    spdw_diag = []
    for m in range(2):
        row = []
        for t_ in range(9):
            tl = wp.tile([96, 96], BF, name=f"spdwd{m}_{t_}")
            sync.dma_start(out=tl, in_=d["spdw_diag"][m, t_])
            row.append(tl)
        spdw_diag.append(row)
    cia_diag = load3("cia_diag")
    cib_diag = load3("cib_diag")
    i96 = load2("i96")
    i48b = load2("i48b")

    dw1_b = [loadb("dw1_b", 96 * m, 96) for m in range(4)]
    dw2_b = [loadb("dw2_b", 96 * m, 96) for m in range(4)]
    cp_in_b = loadb("cp_in_b", 0, C6)
    ci1_b = loadb("ci1_b", 0, C6)
    ci2a_b2 = loadb("ci2a_b2", 0, 128)
    ci2b_b2 = loadb("ci2b_b2", 0, 128)
    ci2c_b = loadb("ci2c_b", 0, C6)
    cp_out_b = [loadb("cp_out_b", 96 * m, 96) for m in range(4)]
    sp_in_b = [loadb("sp_in_b", 96 * m, 96) for m in range(2)]
    sp_dw_b = [loadb("sp_dw_b", 96 * m, 96) for m in range(2)]
    sp_out_b = [loadb("sp_out_b", 96 * m, 96) for m in range(4)]
    proj_b = [loadb("proj_b", 96 * m, 96) for m in range(4)]

    tempb = wp.tile([96, HEADS], F32, name="tempb")
    sync.dma_start(out=tempb, in_=_pbcast(d["temp"][:], 96))

    # ---- persistent activation scratch ------------------------------------
    atp = ctx.enter_context(tc.tile_pool(name="atp", bufs=1))
    at_sb = [atp.tile([96, N], BF, name=f"at{m}") for m in range(4)]
    ap_ = ctx.enter_context(tc.tile_pool(name="ap_", bufs=1))
    ssq = ap_.tile([96, HEADS], F32, name="ssq")
    gscr = ap_.tile([96, 96], BF, name="gscr")
    rn = ap_.tile([96, HEADS], F32, name="rn")
    rnT = ap_.tile([HEADS, 96], F32, name="rnT")
    rqs = ap_.tile([48, HEADS], F32, name="rqs")
    rkb = [ap_.tile([48, 48], F32, name=f"rkb{h}") for h in range(HEADS)]
    ssum = ap_.tile([48, HEADS], F32, name="ssum")
    rs = ap_.tile([48, HEADS], F32, name="rs")
    a_sb = [ap_.tile([48, 48], F32, name=f"a{h}") for h in range(HEADS)]
    en = [ap_.tile([48, 128], BF, name=f"en{p}") for p in range(4)]
    etbd = [ap_.tile([128, 96], BF, name=f"et{p}") for p in range(4)]
    atsum = ap_.tile([96, 4 * NCH], F32, name="atsum")
    cmsum = ap_.tile([96, 4 * NCH], F32, name="cmsum")
    cm_sig = ap_.tile([96, 4], F32, name="cm_sig")

    cxp = ctx.enter_context(tc.tile_pool(name="cxp", bufs=1))
    convx = [cxp.tile([96, N], BF, name=f"cx{m}") for m in range(4)]

    vstack = ExitStack()
    vp = vstack.enter_context(tc.tile_pool(name="vp", bufs=1))
    v_sb = [vp.tile([128, N], BF, name=f"v{m}") for m in range(4)]

    # ==== phase 1: qkv (x streamed per 512-token chunk) + head Grams =======
    with tc.tile_pool(name="xw", bufs=1) as xw, \
         tc.tile_pool(name="xring", bufs=3) as xring, \
         tc.tile_pool(name="qkring", bufs=6) as qkring, \
         tc.tile_pool(name="psQK", bufs=2, space="PSUM") as psQK, \
         tc.tile_pool(name="psV", bufs=2, space="PSUM") as psV, \
         tc.tile_pool(name="psG", bufs=1, space="PSUM") as psG:
        wt_sb = [xw.tile([128, QKW + 512], BF, name=f"wt{k}")
                 for k in range(3)]
        for k in range(3):
            sync.dma_start(out=wt_sb[k], in_=d["wt"][128 * k:128 * k + 128, :])
        gps = psG.tile([96, HEADS * 96], F32, name="gps")

        for cchunk in range(NCH):
            xc = [xring.tile([128, CH], BF, name=f"xc{k}", tag=f"xc{k}")
                  for k in range(3)]
            for k in range(3):
                sync.dma_start(
                    out=xc[k], in_=d["x"][128 * k:128 * k + 128,
                                          CH * cchunk:CH * (cchunk + 1)])
            for j in range(4):
                i = 4 * cchunk + j
                ps = psQK.tile([128, QKW], F32, name="qkps", tag="qkps")
                for o0, ow in ((0, 512), (512, 256)):
                    for k in range(3):
                        nc.tensor.matmul(
                            ps[:, o0:o0 + ow],
                            lhsT=xc[k][:, 128 * j:128 * (j + 1)],
                            rhs=wt_sb[k][:, o0:o0 + ow],
                            start=(k == 0), stop=(k == 2))
                # store head-interleaved: [h0: q48|k48][h1: q48|k48]...
                qkt = qkring.tile([128, QKW], BF, name="qkt", tag="qkt")
                dst = qkt.rearrange("p (h two f) -> p two h f",
                                    two=2, h=HEADS, f=HD)
                srcv = ps.rearrange("p (two h f) -> p two h f",
                                    two=2, h=HEADS, f=HD)
                if i % 2 == 0:
                    nc.scalar.copy(out=dst, in_=srcv)
                else:
                    nc.vector.tensor_copy(out=dst, in_=srcv)
                for h in range(HEADS):
                    lap = qkt[:, 96 * h:96 * (h + 1)]
                    nc.tensor.matmul(
                        gps[:, 96 * h:96 * (h + 1)], lhsT=lap, rhs=lap,
                        start=(i == 0), stop=(i == 4 * NCH - 1),
                        skip_group_check=True)
            for m in range(4):
                ps = psV.tile([128, CH], F32, name="vps", tag="vps")
                for k in range(3):
                    nc.tensor.matmul(
                        ps,
                        lhsT=wt_sb[k][:, QKW + 128 * m:QKW + 128 * (m + 1)],
                        rhs=xc[k],
                        start=(k == 0), stop=(k == 2))
                nc.scalar.copy(
                    out=v_sb[m][:, CH * cchunk:CH * (cchunk + 1)], in_=ps)

        # per-head: diag -> ssq column; copy A block to SBUF
        for h in range(HEADS):
            nc.vector.scalar_tensor_tensor(
                out=gscr, in0=gps[:, 96 * h:96 * (h + 1)], scalar=1.0,
                in1=i96, op0=OP.mult, op1=OP.mult,
                accum_out=ssq[:, h:h + 1])
            nc.vector.tensor_copy(
                out=a_sb[h], in_=gps[0:48, 96 * h + 48:96 * h + 96])

    # ==== phase 3: conv branch dw1 + dw2 (uses v) ==========================
    with tc.tile_pool(name="c1p", bufs=1) as c1pool, \
         tc.tile_pool(name="psC", bufs=6, space="PSUM") as psC:
        conv1p = [c1pool.tile([96, NP1], BF, name=f"c1p{m}") for m in range(4)]
        for m in range(4):
            nc.vector.memset(conv1p[m], 0.0)
        for m in range(4):
            for cchunk in range(NCH):
                ps = psC.tile([96, CH], F32, name="cps", tag="cps")
                for k in range(4):
                    nc.tensor.matmul(
                        ps, lhsT=dw1T[k][:, 96 * m:96 * (m + 1)],
                        rhs=v_sb[k][:, CH * cchunk:CH * (cchunk + 1)],
                        start=(k == 0), stop=(k == 3))
                dst = _win(conv1p[m], (8 * cchunk + 1) * HP + 1,
                           [[HP, 8], [1, W]])
                nc.vector.tensor_scalar(out=dst, in0=ps, scalar1=dw1_b[m],
                                        scalar2=None, op0=OP.add)
        for m in range(4):
            for cchunk in range(NCH):
                ps = psC.tile([96, CH], F32, name="cps2", tag="cps")
                for t_ in range(9):
                    dy, dx = divmod(t_, 3)
                    src = _win(conv1p[m], (8 * cchunk + dy) * HP + dx,
                               [[HP, 8], [1, W]])
                    nc.tensor.matmul(ps, lhsT=dw2_diag[m][t_], rhs=src,
                                     start=(t_ == 0), stop=(t_ == 8))
                nc.vector.tensor_scalar(
                    out=convx[m][:, CH * cchunk:CH * (cchunk + 1)],
                    in0=ps, scalar1=dw2_b[m], scalar2=None, op0=OP.add)


    # ==== phase 2: softmax + at ============================================
    nc.scalar.activation(out=rn, in_=ssq, func=AF.Sqrt)
    nc.vector.reciprocal(out=rn, in_=rn)
    nc.vector.tensor_mul(rqs, rn[0:48, :], tempb[0:48, :])

    with tc.tile_pool(name="psT", bufs=2, space="PSUM") as psT:
        # transpose rn -> rnT so rk values move to the free dim
        psr = psT.tile([HEADS, 96], F32, name="psr", tag="pst")
        nc.tensor.transpose(psr, rn, i96)
        nc.vector.tensor_copy(out=rnT, in_=psr)
        # bounce through DRAM: partition-broadcast DMA needs a DRAM source
        sync.dma_start(out=d["rk_dram"][:], in_=rnT)
        for h in range(HEADS):
            sync.dma_start(out=rkb[h],
                           in_=_pbcast(d["rk_dram"][h:h + 1, 48:96], 48))

        with tc.tile_pool(name="smx", bufs=2) as smx:
            for h in range(HEADS):
                pr, sl = divmod(h, 2)
                as_t = smx.tile([48, 48], F32, name="as_t", tag="as")
                nc.vector.scalar_tensor_tensor(
                    out=as_t, in0=a_sb[h], scalar=rqs[:, h:h + 1],
                    in1=rkb[h], op0=OP.mult, op1=OP.mult)
                mx = smx.tile([48, 1], F32, name="mx", tag="mx")
                nc.vector.tensor_reduce(out=mx, in_=as_t, axis=AX.X,
                                        op=OP.max)
                nc.vector.tensor_scalar_mul(mx, mx, -1.0)
                nc.scalar.activation(
                    out=en[pr][:, 64 * sl:64 * sl + 48], in_=as_t,
                    func=AF.Exp, bias=mx, scale=1.0,
                    accum_out=ssum[:, h:h + 1])
        nc.vector.reciprocal(out=rs, in_=ssum)
        for h in range(HEADS):
            pr, sl = divmod(h, 2)
            sli = en[pr][:, 64 * sl:64 * sl + 48]
            nc.vector.tensor_scalar_mul(sli, sli, rs[:, h:h + 1])

        for pr in range(4):
            nc.vector.memset(en[pr][:, 48:64], 0.0)
            nc.vector.memset(en[pr][:, 112:128], 0.0)
        for pr in range(4):
            nc.vector.memset(etbd[pr], 0.0)
            pst = psT.tile([128, 48], BF, name="pst2", tag="pst2")
            nc.tensor.transpose(pst, en[pr], i48b)
            nc.vector.tensor_copy(out=etbd[pr][0:48, 0:48], in_=pst[0:48, :])
            nc.vector.tensor_copy(out=etbd[pr][64:112, 48:96],
                                  in_=pst[64:112, :])

    with tc.tile_pool(name="psAT", bufs=6, space="PSUM") as psAT:
        for pr in range(4):
            for cchunk in range(NCH):
                ps = psAT.tile([96, CH], F32, name="atps", tag="atps")
                nc.tensor.matmul(
                    ps, lhsT=etbd[pr],
                    rhs=v_sb[pr][:, CH * cchunk:CH * (cchunk + 1)],
                    start=True, stop=True)
                nc.scalar.activation(
                    out=at_sb[pr][:, CH * cchunk:CH * (cchunk + 1)],
                    in_=ps, func=AF.Identity,
                    accum_out=atsum[:, NCH * pr + cchunk:
                                    NCH * pr + cchunk + 1])

    vstack.close()  # release v_sb — last readers are the dw1 matmuls

    # ==== phase 5a: SpatialProjection front (overlaps cp chain) ============
    spstack = ExitStack()
    spl = spstack.enter_context(tc.tile_pool(name="spl", bufs=1))
    psE = spstack.enter_context(tc.tile_pool(name="psE", bufs=2, space="PSUM"))
    sp_pad = [spl.tile([96, NP1], BF, name=f"spp{m}") for m in range(2)]
    for m in range(2):
        nc.vector.memset(sp_pad[m], 0.0)
    for m in range(2):
        for cchunk in range(NCH):
            ps = psE.tile([96, CH], F32, name="eps", tag="eps")
            for k in range(4):
                nc.tensor.matmul(
                    ps, lhsT=spinT[k][:, 96 * m:96 * (m + 1)],
                    rhs=convx[k][:, CH * cchunk:CH * (cchunk + 1)],
                    start=(k == 0), stop=(k == 3))
            dst = _win(sp_pad[m], (8 * cchunk + 1) * HP + 1,
                       [[HP, 8], [1, W]])
            nc.vector.tensor_scalar(out=dst, in0=ps, scalar1=sp_in_b[m],
                                    scalar2=None, op0=OP.add)
    dd = [spl.tile([96, N], BF, name=f"dd{m}") for m in range(2)]
    for m in range(2):
        for cchunk in range(NCH):
            ps = psE.tile([96, CH], F32, name="eps2", tag="eps")
            for t_ in range(9):
                dy, dx = divmod(t_, 3)
                src = _win(sp_pad[m], (8 * cchunk + dy) * HP + dx,
                           [[HP, 8], [1, W]])
                nc.tensor.matmul(ps, lhsT=spdw_diag[m][t_], rhs=src,
                                 start=(t_ == 0), stop=(t_ == 8))
            nc.vector.tensor_scalar(
                out=dd[m][:, CH * cchunk:CH * (cchunk + 1)], in0=ps,
                scalar1=sp_dw_b[m], scalar2=None, op0=OP.add)
    # gg computed in place in dd[0]
    nc.scalar.activation(out=dd[0], in_=dd[0], func=AF.Gelu)
    nc.vector.tensor_mul(dd[0], dd[0], dd[1])

    # ==== phase 4: ChannelProjection on at =================================
    with tc.tile_pool(name="tp", bufs=1) as tp, \
         tc.tile_pool(name="psD", bufs=6, space="PSUM") as psD:
        t_dense = tp.tile([C6, N], BF, name="t_dense")
        tsum = tp.tile([C6, NCH], F32, name="tsum")
        for cchunk in range(NCH):
            ps = psD.tile([C6, CH], F32, name="dps", tag="dps")
            for k in range(4):
                nc.tensor.matmul(
                    ps, lhsT=cpinT[k],
                    rhs=at_sb[k][:, CH * cchunk:CH * (cchunk + 1)],
                    start=(k == 0), stop=(k == 3))
            nc.vector.tensor_scalar(
                out=t_dense[:, CH * cchunk:CH * (cchunk + 1)], in0=ps,
                scalar1=cp_in_b, scalar2=0.0, op0=OP.add, op1=OP.add,
                accum_out=tsum[:, cchunk:cchunk + 1])

        tm = tp.tile([C6, 1], F32, name="tm")
        nc.vector.tensor_reduce(out=tm, in_=tsum, axis=AX.X, op=OP.add)
        nc.vector.tensor_scalar_mul(tm, tm, 1.0 / N)
        ci1v = tp.tile([C6, 1], F32, name="ci1v")
        psc = psD.tile([C6, 1], F32, name="dps1", tag="dps")
        nc.tensor.matmul(psc, lhsT=ci1T, rhs=tm, start=True, stop=True)
        nc.vector.tensor_scalar(out=ci1v, in0=psc, scalar1=ci1_b,
                                scalar2=None, op0=OP.add)

        # packed pad-1 buffer [128 = 2 spatial halves x 64ch, 34*66]
        tpk = tp.tile([128, 34 * HP], BF, name="tpk")
        nc.vector.memset(tpk, 0.0)
        t_img = t_dense.rearrange("p (h w) -> p h w", w=W)
        sync.dma_start(out=_win(tpk, HP + 1, [[HP, 33], [1, W]], p=(0, 64)),
                       in_=t_img[:, 0:33, :])
        sync.dma_start(out=_win(tpk, 1, [[HP, 33], [1, W]], p=(64, 128)),
                       in_=t_img[:, 31:64, :])

        # ci2a (9 taps on PE) -> packed pad-9 buffer [128, 50*82]
        cbp = tp.tile([128, 50 * BP], BF, name="cbp")
        nc.vector.memset(cbp, 0.0)
        for cchunk in range(4):
            ps = psD.tile([128, CH], F32, name="dpsa", tag="dps")
            for t_ in range(9):
                dy, dx = divmod(t_, 3)
                src = _win(tpk, (8 * cchunk + dy) * HP + dx,
                           [[HP, 8], [1, W]])
                nc.tensor.matmul(ps, lhsT=cia_diag[t_], rhs=src,
                                 start=(t_ == 0), stop=(t_ == 8))
            dst = _win(cbp, (8 * cchunk + 9) * BP + 9, [[BP, 8], [1, W]])
            nc.vector.tensor_scalar(out=dst, in0=ps, scalar1=ci2a_b2,
                                    scalar2=None, op0=OP.add)
        # halo exchange between halves
        sync.dma_start(
            out=_win(cbp, 41 * BP + 9, [[BP, 9], [1, W]], p=(0, 64)),
            in_=_win(cbp, 9 * BP + 9, [[BP, 9], [1, W]], p=(64, 128)))
        sync.dma_start(
            out=_win(cbp, 9, [[BP, 9], [1, W]], p=(64, 128)),
            in_=_win(cbp, 32 * BP + 9, [[BP, 9], [1, W]], p=(0, 64)))

        # ci2b 49 dilated taps on PE -> packed dense [128, 2048]
        cbd = tp.tile([128, 2048], BF, name="cbd")
        for cchunk in range(4):
            ps = psD.tile([128, CH], F32, name="dpsb", tag="dps")
            for t_ in range(49):
                ty, tx = divmod(t_, 7)
                src = _win(cbp, (8 * cchunk + 3 * ty) * BP + 3 * tx,
                           [[BP, 8], [1, W]])
                nc.tensor.matmul(ps, lhsT=cib_diag[t_], rhs=src,
                                 start=(t_ == 0), stop=(t_ == 48))
            nc.vector.tensor_scalar(
                out=cbd[:, CH * cchunk:CH * (cchunk + 1)], in0=ps,
                scalar1=ci2b_b2, scalar2=None, op0=OP.add)

        # ci2c via half-masked [128,64] weights -> dense [64, N]
        ci2v = tp.tile([C6, N], BF, name="ci2v")
        for half in range(2):
            for cchunk in range(4):
                ps = psD.tile([C6, CH], F32, name="dpsc", tag="dps")
                nc.tensor.matmul(
                    ps, lhsT=ci2cT[half],
                    rhs=cbd[:, CH * cchunk:CH * (cchunk + 1)],
                    start=True, stop=True)
                nc.vector.tensor_scalar(
                    out=ci2v[:, 2048 * half + CH * cchunk:
                             2048 * half + CH * (cchunk + 1)],
                    in0=ps, scalar1=ci2c_b, scalar2=None, op0=OP.add)

        pprod = tp.tile([C6, N], BF, name="pprod")
        nc.vector.scalar_tensor_tensor(out=pprod, in0=t_dense, scalar=ci1v,
                                       in1=ci2v, op0=OP.mult, op1=OP.mult)

        # cp_out fused: at += channel_map + bias ; per-chunk sums for cm
        for m in range(4):
            for cchunk in range(NCH):
                ps = psD.tile([96, CH], F32, name="dpso", tag="dps")
                nc.tensor.matmul(
                    ps, lhsT=cpoutT[:, 96 * m:96 * (m + 1)],
                    rhs=pprod[:, CH * cchunk:CH * (cchunk + 1)],
                    start=True, stop=True)
                sl = at_sb[m][:, CH * cchunk:CH * (cchunk + 1)]
                nc.vector.scalar_tensor_tensor(
                    out=sl, in0=ps, scalar=cp_out_b[m], in1=sl,
                    op0=OP.add, op1=OP.add,
                    accum_out=cmsum[:, NCH * m + cchunk:
                                    NCH * m + cchunk + 1])
        # cm = (sum(at_new) - sum(at_old)) / N, then sigmoid (scale=1/N)
        red = tp.tile([96, 8], F32, name="red")
        for m in range(4):
            nc.vector.tensor_reduce(out=red[:, 2 * m:2 * m + 1],
                                    in_=cmsum[:, NCH * m:NCH * (m + 1)],
                                    axis=AX.X, op=OP.add)
            nc.vector.tensor_reduce(out=red[:, 2 * m + 1:2 * m + 2],
                                    in_=atsum[:, NCH * m:NCH * (m + 1)],
                                    axis=AX.X, op=OP.add)
            nc.vector.scalar_tensor_tensor(
                out=cm_sig[:, m:m + 1], in0=red[:, 2 * m + 1:2 * m + 2],
                scalar=-1.0, in1=red[:, 2 * m:2 * m + 1],
                op0=OP.mult, op1=OP.add)
        nc.scalar.activation(out=cm_sig, in_=cm_sig, func=AF.Sigmoid,
                             scale=1.0 / N)

    # ==== phase 5b: sp_out + gates (after cp_out updated at) ===============
    gg = dd[0]
    for m in range(4):
        for cchunk in range(NCH):
            ps = psE.tile([96, CH], F32, name="eps3", tag="eps")
            nc.tensor.matmul(
                ps, lhsT=spoutT[:, 96 * m:96 * (m + 1)],
                rhs=gg[:, CH * cchunk:CH * (cchunk + 1)],
                start=True, stop=True)
            sg = spl.tile([96, CH], BF, name="sg", tag="sg", bufs=3)
            nc.scalar.activation(out=sg, in_=ps, func=AF.Sigmoid,
                                 bias=sp_out_b[m], scale=1.0)
            sl = at_sb[m][:, CH * cchunk:CH * (cchunk + 1)]
            nc.vector.tensor_mul(sl, sl, sg)
        nc.vector.scalar_tensor_tensor(
            out=at_sb[m], in0=convx[m], scalar=cm_sig[:, m:m + 1],
            in1=at_sb[m], op0=OP.mult, op1=OP.add)
    spstack.close()

    # ==== phase 6: proj + output ===========================================
    with tc.tile_pool(name="opl", bufs=1) as opl, \
         tc.tile_pool(name="psF", bufs=6, space="PSUM") as psF:
        out_sb = [opl.tile([96, N], F32, name=f"o{m}") for m in range(4)]
        for m in range(4):
            for cchunk in range(NCH):
                ps = psF.tile([96, CH], F32, name="fps", tag="fps")
                for k in range(4):
                    nc.tensor.matmul(
                        ps, lhsT=projT[k][:, 96 * m:96 * (m + 1)],
                        rhs=at_sb[k][:, CH * cchunk:CH * (cchunk + 1)],
                        start=(k == 0), stop=(k == 3))
                nc.vector.tensor_scalar(
                    out=out_sb[m][:, CH * cchunk:CH * (cchunk + 1)], in0=ps,
                    scalar1=proj_b[m], scalar2=None, op0=OP.add)
            sync.dma_start(out=d["out"][96 * m:96 * (m + 1), :],
                           in_=out_sb[m])


def build_program():
    nc = bacc.Bacc("TRN2", target_bir_lowering=False, debug=False,
                   num_devices=NCORES)
    shapes = {
        "wt": (C, QKW + 512, BF), "dw1T": (4, 128, C, BF),
        "projT": (4, 96, C, BF), "cpinT": (4, 96, C6, BF),
        "ci1T": (C6, C6, F32), "ci2cT": (2, 128, C6, BF),
        "cpoutT": (C6, C, BF), "spinT": (4, 96, C2, BF),
        "spoutT": (96, C, BF),
        "dw2_diag": (4, 9, 96, 96, BF), "spdw_diag": (2, 9, 96, 96, BF),
        "cia_diag": (9, 128, 128, BF), "cib_diag": (49, 128, 128, BF),
        "dw1_b": (C, F32), "dw2_b": (C, F32), "cp_in_b": (C6, F32),
        "ci1_b": (C6, F32), "ci2a_b2": (128, F32), "ci2b_b2": (128, F32),
        "ci2c_b": (C6, F32), "cp_out_b": (C, F32), "sp_in_b": (C2, F32),
        "sp_dw_b": (C2, F32), "sp_out_b": (C, F32), "proj_b": (C, F32),
        "temp": (1, HEADS, F32), "i96": (96, 96, F32), "i48b": (48, 48, BF),
    }
    d = {}
    for nm, sh in shapes.items():
        d[nm] = nc.dram_tensor(nm, list(sh[:-1]), sh[-1],
                               kind="ExternalInput")
    d["x"] = nc.dram_tensor("x", [C, N], BF, kind="ExternalInput")
    d["out"] = nc.dram_tensor("out", [C, N], F32, kind="ExternalOutput")
    d["rk_dram"] = nc.dram_tensor("rk_dram", [HEADS, 96], F32)

    with tile.TileContext(nc) as tc:
        emit(tc, d)
    nc.compile()
    return nc


_cached = None


def kernel(**inputs) -> np.ndarray:
    global _last_results, _cached
    x = np.asarray(inputs["x"], np.float32)
    B = x.shape[0]
    assert x.shape == (NCORES, C, H, W), x.shape
    g = build_host_inputs(inputs)
    if _cached is None:
        _cached = build_program()
    nc = _cached

    base = {nm: np.ascontiguousarray(arr) for nm, arr in g.items()}
    in_maps = []
    for b in range(B):
        m = dict(base)
        m["x"] = np.ascontiguousarray(x[b].reshape(C, N)).astype(BF16)
        in_maps.append(m)

    trace = os.environ.get("KERNEL_TRACE") == "1"
    try:
        res = run_bass_kernel_spmd(nc, in_maps, list(range(NCORES)),
                                   trace=trace)
    except ModuleNotFoundError:
        res = run_bass_kernel_spmd(nc, in_maps, list(range(NCORES)),
                                   trace=False)
    _last_results = res
    out = np.stack([res.results[b]["out"].reshape(C, H, W) for b in range(B)])
    return out.astype(np.float32)



# revision 11
# speedup vs baseline: 1.4032x; 1.4032x over previous
"""Trainium2 Bass kernel for nn_Channel_Transposed_Attention (B8 C384 H64 W64).

Data-parallel over batch: 8 batch elements -> 8 NeuronCores (SPMD, per-core
x slice). Per core everything lives in (C, N) channel-major layout
(N = H*W), tiled in 96-channel tiles so attention head-pairs, the x1/x2
gate split and all channel tilings align on partitions. q,k are produced in
(N, C) token-major layout; per head the [q_h|k_h] Gram matrix gives both the
attention logits and the l2-norm diagonals in one accumulated matmul chain.
Depthwise convs run as PE tap-accumulation with per-channel diagonal weight
matrices (bf16) over zero-padded row-strided buffers.
"""
import os
import numpy as np
from contextlib import ExitStack

import concourse.bass as bass
import concourse.bacc as bacc
import concourse.tile as tile
from concourse import mybir
from concourse.bass_utils import run_bass_kernel_spmd
from concourse._compat import with_exitstack

import ml_dtypes
BF16 = ml_dtypes.bfloat16

F32 = mybir.dt.float32
F32R = mybir.dt.float32r
BF = mybir.dt.bfloat16
AF = mybir.ActivationFunctionType
OP = mybir.AluOpType
AX = mybir.AxisListType

H = W = 64
N = H * W               # 4096
HP = W + 2              # 66   pad-1 row stride
NP1 = (H + 2) * HP      # 4356
BP = W + 18             # 82   pad-9 row stride (ci2b)
C = 384
C6, C2, C4 = 64, 192, 96
HEADS, HD = 8, 48
NCORES = 8
CH = 512
NCH = N // CH           # 8
QKW = 2 * C             # 768

_last_results = None


def _r(x):
    return x.bitcast(F32R)


def _win(t, off, dims, p=None):
    """Strided free-dim window of a 2D tile AP at free element offset."""
    base = t[:, off:off + 1] if p is None else t[p[0]:p[1], off:off + 1]
    return bass.AP(tensor=base.tensor, offset=base.offset,
                   ap=[list(base.ap[0])] + [list(dd) for dd in dims])


def _pbcast(row_ap, parts):
    """Partition-broadcast a [1, F] AP to [parts, F]."""
    return bass.AP(tensor=row_ap.tensor, offset=row_ap.offset,
                   ap=[[0, parts]] + [list(dd) for dd in row_ap.ap[1:]])


def _diag(wcol, p):
    d = np.zeros((p, p), np.float32)
    d[np.arange(p), np.arange(p)] = wcol
    return d


def build_host_inputs(inputs):
    g = {}
    qkv_w = np.asarray(inputs["qkv_w"], np.float32)
    wtf = qkv_w.T                                    # [384, 1152]
    # v section padded: pair p -> cols [ch 96p..+48 | 16 zero | ch +48..+96 | 16 zero]
    wtv = np.zeros((C, 512), np.float32)
    for p in range(4):
        wtv[:, 128 * p:128 * p + 48] = wtf[:, QKW + 96 * p:QKW + 96 * p + 48]
        wtv[:, 128 * p + 64:128 * p + 112] = wtf[:, QKW + 96 * p + 48:
                                                 QKW + 96 * (p + 1)]
    g["wt"] = np.ascontiguousarray(
        np.concatenate([wtf[:, :QKW], wtv], 1)).astype(BF16)   # [384, 1280]
    dw1T = np.asarray(inputs["dw1_w"], np.float32).reshape(C, C).T
    dw1Tp = np.zeros((4, 128, C), np.float32)
    for k in range(4):
        dw1Tp[k, 0:48] = dw1T[96 * k:96 * k + 48]
        dw1Tp[k, 64:112] = dw1T[96 * k + 48:96 * (k + 1)]
    g["dw1T"] = np.ascontiguousarray(dw1Tp).astype(BF16)
    g["projT"] = np.ascontiguousarray(
        np.asarray(inputs["proj_w"], np.float32).T.reshape(4, 96, C)
    ).astype(BF16)
    g["cpinT"] = np.ascontiguousarray(
        np.asarray(inputs["cp_in_w"], np.float32).reshape(C6, C).T
        .reshape(4, 96, C6)).astype(BF16)
    g["ci1T"] = np.ascontiguousarray(
        np.asarray(inputs["ci1_w"], np.float32).reshape(C6, C6).T)
    ci2cT = np.asarray(inputs["ci2c_w"], np.float32).reshape(C6, C6).T
    z = np.zeros_like(ci2cT)
    g["ci2cT"] = np.ascontiguousarray(
        np.stack([np.vstack([ci2cT, z]), np.vstack([z, ci2cT])])).astype(BF16)
    g["cpoutT"] = np.ascontiguousarray(
        np.asarray(inputs["cp_out_w"], np.float32).reshape(C, C6).T
    ).astype(BF16)
    g["spinT"] = np.ascontiguousarray(
        np.asarray(inputs["sp_in_w"], np.float32).reshape(C2, C).T
        .reshape(4, 96, C2)).astype(BF16)
    g["spoutT"] = np.ascontiguousarray(
        np.asarray(inputs["sp_out_w"], np.float32).reshape(C, C4).T
    ).astype(BF16)

    # tap vectors for on-device diag construction: [128, 112] fp32
    # cols 0-35: dw2 (4 m-tiles x 9 taps, 96p); 36-53: spdw (2 x 9, 96p);
    # 54-62: cia (9, 128p dup); 63-111: cib (49, 128p dup)
    dw2 = np.asarray(inputs["dw2_w"], np.float32).reshape(C, 9)
    spdw = np.asarray(inputs["sp_dw_w"], np.float32).reshape(C2, 9)
    cia = np.asarray(inputs["ci2a_w"], np.float32).reshape(C6, 9)
    cib = np.asarray(inputs["ci2b_w"], np.float32).reshape(C6, 49)
    taps = np.zeros((128, 112), np.float32)
    for m in range(4):
        for t in range(9):
            taps[0:96, 9 * m + t] = dw2[96 * m:96 * m + 96, t]
    for m in range(2):
        for t in range(9):
            taps[0:96, 36 + 9 * m + t] = spdw[96 * m:96 * m + 96, t]
    for t in range(9):
        taps[0:64, 54 + t] = cia[:, t]
        taps[64:128, 54 + t] = cia[:, t]
    for t in range(49):
        taps[0:64, 63 + t] = cib[:, t]
        taps[64:128, 63 + t] = cib[:, t]
    g["taps"] = np.ascontiguousarray(taps)

    # packed per-partition biases: [128, 29] fp32
    bz = np.zeros((128, 29), np.float32)

    def bput(col, vec, p0=0):
        v = np.asarray(vec, np.float32)
        bz[p0:p0 + v.shape[0], col] = v
    for m in range(4):
        bput(m, inputs["dw1_b"][96 * m:96 * (m + 1)])
        bput(4 + m, inputs["dw2_b"][96 * m:96 * (m + 1)])
        bput(13 + m, inputs["cp_out_b"][96 * m:96 * (m + 1)])
        bput(21 + m, inputs["sp_out_b"][96 * m:96 * (m + 1)])
        bput(25 + m, inputs["proj_b"][96 * m:96 * (m + 1)])
    bput(8, inputs["cp_in_b"])
    bput(9, inputs["ci1_b"])
    bput(10, np.tile(np.asarray(inputs["ci2a_b"], np.float32), 2))
    bput(11, np.tile(np.asarray(inputs["ci2b_b"], np.float32), 2))
    bput(12, inputs["ci2c_b"])
    for m in range(2):
        bput(17 + m, inputs["sp_in_b"][96 * m:96 * (m + 1)])
        bput(19 + m, inputs["sp_dw_b"][96 * m:96 * (m + 1)])
    g["biases"] = np.ascontiguousarray(bz)

    g["temp"] = np.ascontiguousarray(
        np.asarray(inputs["temperature"], np.float32).reshape(1, HEADS))
    g["i96"] = np.eye(96, dtype=np.float32)
    g["i128"] = np.eye(128, dtype=np.float32)
    g["i48b"] = np.eye(48, dtype=np.float32).astype(BF16)
    sel = np.zeros((8, 384), np.float32)
    for h in range(8):
        sel[h, 48 * h:48 * (h + 1)] = 1.0
    g["sel8"] = np.ascontiguousarray(sel)
    return g


@with_exitstack
def emit(ctx: ExitStack, tc, d):
    nc = tc.nc
    sync = nc.sync

    # ---- persistent weights ------------------------------------------------
    wp = ctx.enter_context(tc.tile_pool(name="wp", bufs=1))

    def load2(nm):
        src = d[nm]
        t = wp.tile(list(src.shape), src.dtype, name=f"sb_{nm}")
        sync.dma_start(out=t, in_=src[:])
        return t

    def load3(nm):
        src = d[nm]
        ts = []
        for i in range(src.shape[0]):
            t = wp.tile(list(src.shape[1:]), src.dtype, name=f"sb_{nm}{i}")
            sync.dma_start(out=t, in_=src[i])
            ts.append(t)
        return ts

    # small constants first (identities, tap vectors, packed biases)
    i96 = load2("i96")
    i128 = load2("i128")
    i48b = load2("i48b")
    taps_sb = load2("taps")
    bs = load2("biases")
    sel8 = load2("sel8")
    tempb = wp.tile([96, HEADS], F32, name="tempb")
    sync.dma_start(out=tempb, in_=_pbcast(d["temp"][:], 96))

    dw1_b = [bs[0:96, m:m + 1] for m in range(4)]
    dw2_b = [bs[0:96, 4 + m:5 + m] for m in range(4)]
    cp_in_b = bs[0:C6, 8:9]
    ci1_b = bs[0:C6, 9:10]
    ci2a_b2 = bs[0:128, 10:11]
    ci2b_b2 = bs[0:128, 11:12]
    ci2c_b = bs[0:C6, 12:13]
    cp_out_b = [bs[0:96, 13 + m:14 + m] for m in range(4)]
    sp_in_b = [bs[0:96, 17 + m:18 + m] for m in range(2)]
    sp_dw_b = [bs[0:96, 19 + m:20 + m] for m in range(2)]
    sp_out_b = [bs[0:96, 21 + m:22 + m] for m in range(4)]
    proj_b = [bs[0:96, 25 + m:26 + m] for m in range(4)]

    # diag weight tiles, built on-device from tap vectors (no DMA storm);
    # the actual builds are interleaved into the phase-1 chunk loop
    dw2_diag = [[wp.tile([96, 96], BF, name=f"dw2d{m}_{t_}")
                 for t_ in range(9)] for m in range(4)]
    spdw_diag = [[wp.tile([96, 96], BF, name=f"spdwd{m}_{t_}")
                  for t_ in range(9)] for m in range(2)]
    cia_diag = [wp.tile([128, 128], BF, name=f"ciad{t_}") for t_ in range(9)]
    cib_diag = [wp.tile([128, 128], BF, name=f"cibd{t_}") for t_ in range(49)]
    diag_jobs = []
    for m in range(4):
        for t_ in range(9):
            diag_jobs.append((dw2_diag[m][t_], i96, 9 * m + t_, 96))
    for m in range(2):
        for t_ in range(9):
            diag_jobs.append((spdw_diag[m][t_], i96, 36 + 9 * m + t_, 96))
    for t_ in range(9):
        diag_jobs.append((cia_diag[t_], i128, 54 + t_, 128))
    for t_ in range(49):
        diag_jobs.append((cib_diag[t_], i128, 63 + t_, 128))

    def emit_diag(n):
        while n > 0 and diag_jobs:
            dst, ident, col, p = diag_jobs.pop(0)
            nc.vector.tensor_scalar(
                out=dst, in0=ident[0:p, 0:p], scalar1=taps_sb[0:p, col:col + 1],
                scalar2=None, op0=OP.mult)
            n -= 1

    # ---- persistent activation scratch ------------------------------------
    atp = ctx.enter_context(tc.tile_pool(name="atp", bufs=1))
    at_sb = [atp.tile([96, N], BF, name=f"at{m}") for m in range(4)]
    ap_ = ctx.enter_context(tc.tile_pool(name="ap_", bufs=1))
    ssq = ap_.tile([96, HEADS], F32, name="ssq")
    gscr = ap_.tile([96, 96], BF, name="gscr")
    rn = ap_.tile([96, HEADS], F32, name="rn")
    rnT = ap_.tile([HEADS, 96], F32, name="rnT")
    rqs = ap_.tile([48, HEADS], F32, name="rqs")
    rkb = [ap_.tile([48, 48], F32, name=f"rkb{h}") for h in range(HEADS)]
    ssum = ap_.tile([48, HEADS], F32, name="ssum")
    rs = ap_.tile([48, HEADS], F32, name="rs")
    a_sb = [ap_.tile([48, 48], F32, name=f"a{h}") for h in range(HEADS)]
    en = [ap_.tile([48, 128], BF, name=f"en{p}") for p in range(4)]
    etbd = [ap_.tile([128, 96], BF, name=f"et{p}") for p in range(4)]
    atsum = ap_.tile([96, 4 * NCH], F32, name="atsum")
    cmsum = ap_.tile([96, 4 * NCH], F32, name="cmsum")
    cm_sig = ap_.tile([96, 4], F32, name="cm_sig")

    cxp = ctx.enter_context(tc.tile_pool(name="cxp", bufs=1))
    convx = [cxp.tile([96, N], BF, name=f"cx{m}") for m in range(4)]

    vstack = ExitStack()
    vp = vstack.enter_context(tc.tile_pool(name="vp", bufs=1))
    v_sb = [vp.tile([128, N], BF, name=f"v{m}") for m in range(4)]

    # ==== phase 1: qkv (x streamed per 512-token chunk) + head Grams =======
    with tc.tile_pool(name="xw", bufs=1) as xw, \
         tc.tile_pool(name="xring", bufs=3) as xring, \
         tc.tile_pool(name="qkring", bufs=6) as qkring, \
         tc.tile_pool(name="psQK", bufs=2, space="PSUM") as psQK, \
         tc.tile_pool(name="psV", bufs=2, space="PSUM") as psV, \
         tc.tile_pool(name="psG", bufs=1, space="PSUM") as psG:
        wt_sb = [xw.tile([128, QKW + 512], BF, name=f"wt{k}")
                 for k in range(3)]
        for k in range(3):
            sync.dma_start(out=wt_sb[k], in_=d["wt"][128 * k:128 * k + 128, :])
        gps = psG.tile([96, HEADS * 96], F32, name="gps")

        for cchunk in range(NCH):
            xc = [xring.tile([128, CH], BF, name=f"xc{k}", tag=f"xc{k}")
                  for k in range(3)]
            for k in range(3):
                sync.dma_start(
                    out=xc[k], in_=d["x"][128 * k:128 * k + 128,
                                          CH * cchunk:CH * (cchunk + 1)])
            for j in range(4):
                i = 4 * cchunk + j
                ps = psQK.tile([128, QKW], F32, name="qkps", tag="qkps")
                for o0, ow in ((0, 512), (512, 256)):
                    for k in range(3):
                        nc.tensor.matmul(
                            ps[:, o0:o0 + ow],
                            lhsT=xc[k][:, 128 * j:128 * (j + 1)],
                            rhs=wt_sb[k][:, o0:o0 + ow],
                            start=(k == 0), stop=(k == 2))
                # store head-interleaved: [h0: q48|k48][h1: q48|k48]...
                qkt = qkring.tile([128, QKW], BF, name="qkt", tag="qkt")
                dst = qkt.rearrange("p (h two f) -> p two h f",
                                    two=2, h=HEADS, f=HD)
                srcv = ps.rearrange("p (two h f) -> p two h f",
                                    two=2, h=HEADS, f=HD)
                if i % 2 == 0:
                    nc.scalar.copy(out=dst, in_=srcv)
                else:
                    nc.vector.tensor_copy(out=dst, in_=srcv)
                for h in range(HEADS):
                    lap = qkt[:, 96 * h:96 * (h + 1)]
                    nc.tensor.matmul(
                        gps[:, 96 * h:96 * (h + 1)], lhsT=lap, rhs=lap,
                        start=(i == 0), stop=(i == 4 * NCH - 1),
                        skip_group_check=True)
            for m in range(4):
                ps = psV.tile([128, CH], F32, name="vps", tag="vps")
                for k in range(3):
                    nc.tensor.matmul(
                        ps,
                        lhsT=wt_sb[k][:, QKW + 128 * m:QKW + 128 * (m + 1)],
                        rhs=xc[k],
                        start=(k == 0), stop=(k == 2))
                nc.scalar.copy(
                    out=v_sb[m][:, CH * cchunk:CH * (cchunk + 1)], in_=ps)
            emit_diag(14)

        # per-head: diag -> ssq column; copy A block to SBUF
        for h in range(HEADS):
            nc.vector.scalar_tensor_tensor(
                out=gscr, in0=gps[:, 96 * h:96 * (h + 1)], scalar=1.0,
                in1=i96, op0=OP.mult, op1=OP.mult,
                accum_out=ssq[:, h:h + 1])
            nc.vector.tensor_copy(
                out=a_sb[h], in_=gps[0:48, 96 * h + 48:96 * h + 96])
        emit_diag(len(diag_jobs))

    # big weights for the later phases (emitted after phase 1 so their DMAs
    # queue behind the x/wt loads the PE needs first)
    dw1T = load3("dw1T")
    projT = load3("projT")
    cpinT = load3("cpinT")
    ci1T = load2("ci1T")
    ci2cT = load3("ci2cT")
    cpoutT = load2("cpoutT")
    spinT = load3("spinT")
    spoutT = load2("spoutT")

    # ==== phase 3: conv branch dw1 + dw2 (uses v) ==========================
    with tc.tile_pool(name="c1p", bufs=1) as c1pool, \
         tc.tile_pool(name="psC", bufs=6, space="PSUM") as psC:
        conv1p = [c1pool.tile([96, NP1], BF, name=f"c1p{m}") for m in range(4)]
        for m in range(4):
            nc.vector.memset(conv1p[m], 0.0)
        for m in range(4):
            for cchunk in range(NCH):
                ps = psC.tile([96, CH], F32, name="cps", tag="cps")
                for k in range(4):
                    nc.tensor.matmul(
                        ps, lhsT=dw1T[k][:, 96 * m:96 * (m + 1)],
                        rhs=v_sb[k][:, CH * cchunk:CH * (cchunk + 1)],
                        start=(k == 0), stop=(k == 3))
                dst = _win(conv1p[m], (8 * cchunk + 1) * HP + 1,
                           [[HP, 8], [1, W]])
                nc.vector.tensor_scalar(out=dst, in0=ps, scalar1=dw1_b[m],
                                        scalar2=None, op0=OP.add)
        for m in range(4):
            for cchunk in range(NCH):
                ps = psC.tile([96, CH], F32, name="cps2", tag="cps")
                for t_ in range(9):
                    dy, dx = divmod(t_, 3)
                    src = _win(conv1p[m], (8 * cchunk + dy) * HP + dx,
                               [[HP, 8], [1, W]])
                    nc.tensor.matmul(ps, lhsT=dw2_diag[m][t_], rhs=src,
                                     start=(t_ == 0), stop=(t_ == 8))
                nc.vector.tensor_scalar(
                    out=convx[m][:, CH * cchunk:CH * (cchunk + 1)],
                    in0=ps, scalar1=dw2_b[m], scalar2=None, op0=OP.add)


    # ==== phase 2: softmax + at ============================================
    nc.scalar.activation(out=rn, in_=ssq, func=AF.Sqrt)
    nc.vector.reciprocal(out=rn, in_=rn)
    nc.vector.tensor_mul(rqs, rn[0:48, :], tempb[0:48, :])

    with tc.tile_pool(name="psT", bufs=2, space="PSUM") as psT:
        # transpose rn -> rnT so rk values move to the free dim
        psr = psT.tile([HEADS, 96], F32, name="psr", tag="pst")
        nc.tensor.transpose(psr, rn, i96)
        nc.vector.tensor_copy(out=rnT, in_=psr)
        # partition-broadcast rk rows via selector matmul (contract over the
        # 8 partitions of rnT; sel8 row h carries ones in cols 48h..48h+48)
        for h in range(HEADS):
            psb = psT.tile([48, 48], F32, name="psb", tag="psb")
            nc.tensor.matmul(psb, lhsT=sel8[0:8, 48 * h:48 * (h + 1)],
                             rhs=rnT[0:8, 48:96], start=True, stop=True)
            nc.vector.tensor_copy(out=rkb[h], in_=psb)

        with tc.tile_pool(name="smx", bufs=2) as smx:
            for h in range(HEADS):
                pr, sl = divmod(h, 2)
                as_t = smx.tile([48, 48], F32, name="as_t", tag="as")
                nc.vector.scalar_tensor_tensor(
                    out=as_t, in0=a_sb[h], scalar=rqs[:, h:h + 1],
                    in1=rkb[h], op0=OP.mult, op1=OP.mult)
                mx = smx.tile([48, 1], F32, name="mx", tag="mx")
                nc.vector.tensor_reduce(out=mx, in_=as_t, axis=AX.X,
                                        op=OP.max)
                nc.vector.tensor_scalar_mul(mx, mx, -1.0)
                nc.scalar.activation(
                    out=en[pr][:, 64 * sl:64 * sl + 48], in_=as_t,
                    func=AF.Exp, bias=mx, scale=1.0,
                    accum_out=ssum[:, h:h + 1])
        nc.vector.reciprocal(out=rs, in_=ssum)
        for h in range(HEADS):
            pr, sl = divmod(h, 2)
            sli = en[pr][:, 64 * sl:64 * sl + 48]
            nc.vector.tensor_scalar_mul(sli, sli, rs[:, h:h + 1])

        for pr in range(4):
            nc.vector.memset(en[pr][:, 48:64], 0.0)
            nc.vector.memset(en[pr][:, 112:128], 0.0)
        for pr in range(4):
            nc.vector.memset(etbd[pr], 0.0)
            pst = psT.tile([128, 48], BF, name="pst2", tag="pst2")
            nc.tensor.transpose(pst, en[pr], i48b)
            nc.vector.tensor_copy(out=etbd[pr][0:48, 0:48], in_=pst[0:48, :])
            nc.vector.tensor_copy(out=etbd[pr][64:112, 48:96],
                                  in_=pst[64:112, :])

    with tc.tile_pool(name="psAT", bufs=6, space="PSUM") as psAT:
        for pr in range(4):
            for cchunk in range(NCH):
                ps = psAT.tile([96, CH], F32, name="atps", tag="atps")
                nc.tensor.matmul(
                    ps, lhsT=etbd[pr],
                    rhs=v_sb[pr][:, CH * cchunk:CH * (cchunk + 1)],
                    start=True, stop=True)
                nc.scalar.activation(
                    out=at_sb[pr][:, CH * cchunk:CH * (cchunk + 1)],
                    in_=ps, func=AF.Identity,
                    accum_out=atsum[:, NCH * pr + cchunk:
                                    NCH * pr + cchunk + 1])

    vstack.close()  # release v_sb — last readers are the dw1 matmuls

    # ==== phase 5a: SpatialProjection front (overlaps cp chain) ============
    spstack = ExitStack()
    spl = spstack.enter_context(tc.tile_pool(name="spl", bufs=1))
    psE = spstack.enter_context(tc.tile_pool(name="psE", bufs=2, space="PSUM"))
    sp_pad = [spl.tile([96, NP1], BF, name=f"spp{m}") for m in range(2)]
    for m in range(2):
        nc.vector.memset(sp_pad[m], 0.0)
    for m in range(2):
        for cchunk in range(NCH):
            ps = psE.tile([96, CH], F32, name="eps", tag="eps")
            for k in range(4):
                nc.tensor.matmul(
                    ps, lhsT=spinT[k][:, 96 * m:96 * (m + 1)],
                    rhs=convx[k][:, CH * cchunk:CH * (cchunk + 1)],
                    start=(k == 0), stop=(k == 3))
            dst = _win(sp_pad[m], (8 * cchunk + 1) * HP + 1,
                       [[HP, 8], [1, W]])
            nc.vector.tensor_scalar(out=dst, in0=ps, scalar1=sp_in_b[m],
                                    scalar2=None, op0=OP.add)
    dd = [spl.tile([96, N], BF, name=f"dd{m}") for m in range(2)]
    for m in range(2):
        for cchunk in range(NCH):
            ps = psE.tile([96, CH], F32, name="eps2", tag="eps")
            for t_ in range(9):
                dy, dx = divmod(t_, 3)
                src = _win(sp_pad[m], (8 * cchunk + dy) * HP + dx,
                           [[HP, 8], [1, W]])
                nc.tensor.matmul(ps, lhsT=spdw_diag[m][t_], rhs=src,
                                 start=(t_ == 0), stop=(t_ == 8))
            nc.vector.tensor_scalar(
                out=dd[m][:, CH * cchunk:CH * (cchunk + 1)], in0=ps,
                scalar1=sp_dw_b[m], scalar2=None, op0=OP.add)
    # gg computed in place in dd[0]
    nc.scalar.activation(out=dd[0], in_=dd[0], func=AF.Gelu)
    nc.vector.tensor_mul(dd[0], dd[0], dd[1])

    # ==== phase 4: ChannelProjection on at =================================
    with tc.tile_pool(name="tp", bufs=1) as tp, \
         tc.tile_pool(name="psD", bufs=6, space="PSUM") as psD:
        t_dense = tp.tile([C6, N], BF, name="t_dense")
        tsum = tp.tile([C6, NCH], F32, name="tsum")
        for cchunk in range(NCH):
            ps = psD.tile([C6, CH], F32, name="dps", tag="dps")
            for k in range(4):
                nc.tensor.matmul(
                    ps, lhsT=cpinT[k],
                    rhs=at_sb[k][:, CH * cchunk:CH * (cchunk + 1)],
                    start=(k == 0), stop=(k == 3))
            nc.vector.tensor_scalar(
                out=t_dense[:, CH * cchunk:CH * (cchunk + 1)], in0=ps,
                scalar1=cp_in_b, scalar2=0.0, op0=OP.add, op1=OP.add,
                accum_out=tsum[:, cchunk:cchunk + 1])

        tm = tp.tile([C6, 1], F32, name="tm")
        nc.vector.tensor_reduce(out=tm, in_=tsum, axis=AX.X, op=OP.add)
        nc.vector.tensor_scalar_mul(tm, tm, 1.0 / N)
        ci1v = tp.tile([C6, 1], F32, name="ci1v")
        psc = psD.tile([C6, 1], F32, name="dps1", tag="dps")
        nc.tensor.matmul(psc, lhsT=ci1T, rhs=tm, start=True, stop=True)
        nc.vector.tensor_scalar(out=ci1v, in0=psc, scalar1=ci1_b,
                                scalar2=None, op0=OP.add)

        # packed pad-1 buffer [128 = 2 spatial halves x 64ch, 34*66]
        tpk = tp.tile([128, 34 * HP], BF, name="tpk")
        nc.vector.memset(tpk, 0.0)
        t_img = t_dense.rearrange("p (h w) -> p h w", w=W)
        sync.dma_start(out=_win(tpk, HP + 1, [[HP, 33], [1, W]], p=(0, 64)),
                       in_=t_img[:, 0:33, :])
        sync.dma_start(out=_win(tpk, 1, [[HP, 33], [1, W]], p=(64, 128)),
                       in_=t_img[:, 31:64, :])

        # ci2a (9 taps on PE) -> packed pad-9 buffer [128, 50*82]
        cbp = tp.tile([128, 50 * BP], BF, name="cbp")
        nc.vector.memset(cbp, 0.0)
        for cchunk in range(4):
            ps = psD.tile([128, CH], F32, name="dpsa", tag="dps")
            for t_ in range(9):
                dy, dx = divmod(t_, 3)
                src = _win(tpk, (8 * cchunk + dy) * HP + dx,
                           [[HP, 8], [1, W]])
                nc.tensor.matmul(ps, lhsT=cia_diag[t_], rhs=src,
                                 start=(t_ == 0), stop=(t_ == 8))
            dst = _win(cbp, (8 * cchunk + 9) * BP + 9, [[BP, 8], [1, W]])
            nc.vector.tensor_scalar(out=dst, in0=ps, scalar1=ci2a_b2,
                                    scalar2=None, op0=OP.add)
        # halo exchange between halves
        sync.dma_start(
            out=_win(cbp, 41 * BP + 9, [[BP, 9], [1, W]], p=(0, 64)),
            in_=_win(cbp, 9 * BP + 9, [[BP, 9], [1, W]], p=(64, 128)))
        sync.dma_start(
            out=_win(cbp, 9, [[BP, 9], [1, W]], p=(64, 128)),
            in_=_win(cbp, 32 * BP + 9, [[BP, 9], [1, W]], p=(0, 64)))

        # ci2b 49 dilated taps on PE -> packed dense [128, 2048]
        cbd = tp.tile([128, 2048], BF, name="cbd")
        for cchunk in range(4):
            ps = psD.tile([128, CH], F32, name="dpsb", tag="dps")
            for t_ in range(49):
                ty, tx = divmod(t_, 7)
                src = _win(cbp, (8 * cchunk + 3 * ty) * BP + 3 * tx,
                           [[BP, 8], [1, W]])
                nc.tensor.matmul(ps, lhsT=cib_diag[t_], rhs=src,
                                 start=(t_ == 0), stop=(t_ == 48))
            nc.vector.tensor_scalar(
                out=cbd[:, CH * cchunk:CH * (cchunk + 1)], in0=ps,
                scalar1=ci2b_b2, scalar2=None, op0=OP.add)

        # ci2c via half-masked [128,64] weights -> dense [64, N]
        ci2v = tp.tile([C6, N], BF, name="ci2v")
        for half in range(2):
            for cchunk in range(4):
                ps = psD.tile([C6, CH], F32, name="dpsc", tag="dps")
                nc.tensor.matmul(
                    ps, lhsT=ci2cT[half],
                    rhs=cbd[:, CH * cchunk:CH * (cchunk + 1)],
                    start=True, stop=True)
                nc.vector.tensor_scalar(
                    out=ci2v[:, 2048 * half + CH * cchunk:
                             2048 * half + CH * (cchunk + 1)],
                    in0=ps, scalar1=ci2c_b, scalar2=None, op0=OP.add)

        pprod = tp.tile([C6, N], BF, name="pprod")
        nc.vector.scalar_tensor_tensor(out=pprod, in0=t_dense, scalar=ci1v,
                                       in1=ci2v, op0=OP.mult, op1=OP.mult)

        # cp_out fused: at += channel_map + bias ; per-chunk sums for cm
        for m in range(4):
            for cchunk in range(NCH):
                ps = psD.tile([96, CH], F32, name="dpso", tag="dps")
                nc.tensor.matmul(
                    ps, lhsT=cpoutT[:, 96 * m:96 * (m + 1)],
                    rhs=pprod[:, CH * cchunk:CH * (cchunk + 1)],
                    start=True, stop=True)
                sl = at_sb[m][:, CH * cchunk:CH * (cchunk + 1)]
                nc.vector.scalar_tensor_tensor(
                    out=sl, in0=ps, scalar=cp_out_b[m], in1=sl,
                    op0=OP.add, op1=OP.add,
                    accum_out=cmsum[:, NCH * m + cchunk:
                                    NCH * m + cchunk + 1])
        # cm = (sum(at_new) - sum(at_old)) / N, then sigmoid (scale=1/N)
        red = tp.tile([96, 8], F32, name="red")
        for m in range(4):
            nc.vector.tensor_reduce(out=red[:, 2 * m:2 * m + 1],
                                    in_=cmsum[:, NCH * m:NCH * (m + 1)],
                                    axis=AX.X, op=OP.add)
            nc.vector.tensor_reduce(out=red[:, 2 * m + 1:2 * m + 2],
                                    in_=atsum[:, NCH * m:NCH * (m + 1)],
                                    axis=AX.X, op=OP.add)
            nc.vector.scalar_tensor_tensor(
                out=cm_sig[:, m:m + 1], in0=red[:, 2 * m + 1:2 * m + 2],
                scalar=-1.0, in1=red[:, 2 * m:2 * m + 1],
                op0=OP.mult, op1=OP.add)
        nc.scalar.activation(out=cm_sig, in_=cm_sig, func=AF.Sigmoid,
                             scale=1.0 / N)

    # ==== phase 5b: sp_out + gates (after cp_out updated at) ===============
    gg = dd[0]
    for m in range(4):
        for cchunk in range(NCH):
            ps = psE.tile([96, CH], F32, name="eps3", tag="eps")
            nc.tensor.matmul(
                ps, lhsT=spoutT[:, 96 * m:96 * (m + 1)],
                rhs=gg[:, CH * cchunk:CH * (cchunk + 1)],
                start=True, stop=True)
            sg = spl.tile([96, CH], BF, name="sg", tag="sg", bufs=3)
            nc.scalar.activation(out=sg, in_=ps, func=AF.Sigmoid,
                                 bias=sp_out_b[m], scale=1.0)
            sl = at_sb[m][:, CH * cchunk:CH * (cchunk + 1)]
            nc.vector.tensor_mul(sl, sl, sg)
        nc.vector.scalar_tensor_tensor(
            out=at_sb[m], in0=convx[m], scalar=cm_sig[:, m:m + 1],
            in1=at_sb[m], op0=OP.mult, op1=OP.add)
    spstack.close()

    # ==== phase 6: proj + output ===========================================
    with tc.tile_pool(name="opl", bufs=1) as opl, \
         tc.tile_pool(name="psF", bufs=6, space="PSUM") as psF:
        out_sb = [opl.tile([96, N], F32, name=f"o{m}") for m in range(4)]
        for m in range(4):
            for cchunk in range(NCH):
                ps = psF.tile([96, CH], F32, name="fps", tag="fps")
                for k in range(4):
                    nc.tensor.matmul(
                        ps, lhsT=projT[k][:, 96 * m:96 * (m + 1)],
                        rhs=at_sb[k][:, CH * cchunk:CH * (cchunk + 1)],
                        start=(k == 0), stop=(k == 3))
                nc.vector.tensor_scalar(
                    out=out_sb[m][:, CH * cchunk:CH * (cchunk + 1)], in0=ps,
                    scalar1=proj_b[m], scalar2=None, op0=OP.add)
            sync.dma_start(out=d["out"][96 * m:96 * (m + 1), :],
                           in_=out_sb[m])


def build_program():
    nc = bacc.Bacc("TRN2", target_bir_lowering=False, debug=False,
                   num_devices=NCORES)
    shapes = {
        "wt": (C, QKW + 512, BF), "dw1T": (4, 128, C, BF),
        "projT": (4, 96, C, BF), "cpinT": (4, 96, C6, BF),
        "ci1T": (C6, C6, F32), "ci2cT": (2, 128, C6, BF),
        "cpoutT": (C6, C, BF), "spinT": (4, 96, C2, BF),
        "spoutT": (96, C, BF),
        "taps": (128, 112, F32), "biases": (128, 29, F32),
        "sel8": (8, 384, F32),
        "temp": (1, HEADS, F32), "i96": (96, 96, F32),
        "i128": (128, 128, F32), "i48b": (48, 48, BF),
    }
    d = {}
    for nm, sh in shapes.items():
        d[nm] = nc.dram_tensor(nm, list(sh[:-1]), sh[-1],
                               kind="ExternalInput")
    d["x"] = nc.dram_tensor("x", [C, N], BF, kind="ExternalInput")
    d["out"] = nc.dram_tensor("out", [C, N], F32, kind="ExternalOutput")

    with tile.TileContext(nc) as tc:
        emit(tc, d)
    nc.compile()
    return nc


_cached = None


def kernel(**inputs) -> np.ndarray:
    global _last_results, _cached
    x = np.asarray(inputs["x"], np.float32)
    B = x.shape[0]
    assert x.shape == (NCORES, C, H, W), x.shape
    g = build_host_inputs(inputs)
    if _cached is None:
        _cached = build_program()
    nc = _cached

    base = {nm: np.ascontiguousarray(arr) for nm, arr in g.items()}
    in_maps = []
    for b in range(B):
        m = dict(base)
        m["x"] = np.ascontiguousarray(x[b].reshape(C, N)).astype(BF16)
        in_maps.append(m)

    trace = os.environ.get("KERNEL_TRACE") == "1"
    try:
        res = run_bass_kernel_spmd(nc, in_maps, list(range(NCORES)),
                                   trace=trace)
    except ModuleNotFoundError:
        res = run_bass_kernel_spmd(nc, in_maps, list(range(NCORES)),
                                   trace=False)
    _last_results = res
    out = np.stack([res.results[b]["out"].reshape(C, H, W) for b in range(B)])
    return out.astype(np.float32)

